# revision 1
# baseline (speedup 1.0000x reference)
"""AtomAttentionDecoder — 8-core Trainium2, single SPMD executable.

Same math/sharding as the baseline (batch(2) x 4 window-slices, halo 8
windows), but all 8 shards run as ONE shard_map'd jit program instead of 8
separate per-device executables: one compile, and the shards execute
concurrently instead of serializing through the dispatch tunnel.
"""

import numpy as np

B, NTOK, NATOM = 2, 512, 16384
C_TOKEN, C_ATOM, C_PAIR, C_S = 384, 128, 16, 384
NQ, NK, H, NB = 32, 128, 4, 3
DH = C_ATOM // H
NW = NATOM // NQ

WSLICES = 4
KEPT_W = NW // WSLICES      # 128
HALO_W = 8
LOC_W = KEPT_W + 2 * HALO_W  # 144
KEPT_A = KEPT_W * NQ        # 4096
HALO_A = HALO_W * NQ        # 256
LOC_A = LOC_W * NQ          # 4608

NCORES = 8

_jitted = None

WNAMES = ['Wa', 'lnq_g', 'lnq_b', 'Wout',
          'ag_w', 'ag_b', 'ab_w', 'wq', 'bq', 'wk', 'wv', 'pg', 'pb',
          'wpb', 'wg', 'wo', 'sk_w', 'sk_b', 'tg_w', 'tg_b', 'tb_w',
          'wt1', 'wt2', 'wto', 'tk_w', 'tk_b']


def _build_spmd_fn():
    import jax
    import jax.numpy as jnp
    from jax.sharding import Mesh, PartitionSpec
    from jax.experimental.shard_map import shard_map

    def _ln(x, g=None, b=None, eps=1e-5):
        mu = jnp.mean(x, -1, keepdims=True)
        var = jnp.mean((x - mu) ** 2, -1, keepdims=True)
        xn = (x - mu) * jax.lax.rsqrt(var + eps)
        if g is not None:
            xn = xn * g
        if b is not None:
            xn = xn + b
        return xn

    def body(a, ef, plm, am, idx, *ws):
        bf = jnp.bfloat16
        def mm(x, w):
            return jnp.matmul(x.astype(bf), w.astype(bf),
                              preferred_element_type=jnp.float32)

        # weights arrive stacked [1, ...] per core — drop the shard dim
        (Wa, lnq_g, lnq_b, Wout,
         ag_w, ag_b, ab_w, wq, bq, wk, wv, pg, pb, wpb, wg, wo,
         sk_w, sk_b, tg_w, tg_b, tb_w, wt1, wt2, wto, tk_w, tk_b) = [
            w[0] for w in ws]

        q = mm(a, Wa)                                    # [NTOK, C_ATOM]
        q = jnp.take(q, idx, axis=0)                  # [LOC_A, C_ATOM]
        q = q + ef
        amc = am[:, None]
        q = q * amc
        s = jnp.pad(ef, ((0, 0), (0, C_S - C_ATOM)))
        sn = _ln(s)

        NBLK = NK // NQ  # 4

        def windows(t):
            pad = [(48, 80)] + [(0, 0)] * (t.ndim - 1)
            tp = jnp.pad(t, pad)
            blk = tp.reshape((LOC_W + NBLK, NQ) + t.shape[1:])
            w = jnp.stack([blk[j:j + LOC_W] for j in range(NBLK)], axis=1)
            return w.reshape((LOC_W, NK) + t.shape[1:])

        keymask = windows(am)                          # [LOC_W, NK]

        x = q
        inv = 1.0 / np.sqrt(DH)
        for i in range(NB):
            xa = jax.nn.sigmoid(mm(sn, ag_w[i]) + ag_b[i]) * _ln(x) + mm(sn, ab_w[i])
            qh = (mm(xa, wq[i]) + bq[i]).reshape(LOC_W, NQ, H, DH)
            kh = (mm(xa, wk[i])).reshape(LOC_A, H, DH)
            vh = (mm(xa, wv[i])).reshape(LOC_A, H, DH)
            kw = windows(kh)                                   # [LOC_W, NK, H, DH]
            vw = windows(vh)
            bias = mm(_ln(plm, pg[i], pb[i]), wpb[i])             # [LOC_W, NQ, NK, H]
            scores = jnp.einsum('wqhd,wkhd->wqkh', qh.astype(bf), kw.astype(bf), preferred_element_type=jnp.float32) * inv + bias
            scores = jnp.where(keymask[:, None, :, None] > 0, scores, -1e9)
            attn = jax.nn.softmax(scores, axis=2)
            o = jnp.einsum('wqkh,wkhd->wqhd', attn.astype(bf), vw.astype(bf), preferred_element_type=jnp.float32).reshape(LOC_A, C_ATOM)
            gate = jax.nn.sigmoid(mm(xa, wg[i]))
            x = x + jax.nn.sigmoid(mm(sn, sk_w[i]) + sk_b[i]) * mm(gate * o, wo[i])
            xt = jax.nn.sigmoid(mm(sn, tg_w[i]) + tg_b[i]) * _ln(x) + mm(sn, tb_w[i])
            hsw = jax.nn.silu(mm(xt, wt1[i])) * mm(xt, wt2[i])
            x = x + jax.nn.sigmoid(mm(sn, tk_w[i]) + tk_b[i]) * mm(hsw, wto[i])

        x = x * amc
        r = _ln(x, lnq_g, lnq_b) @ Wout                        # [LOC_A, 3]
        return r[HALO_A:HALO_A + KEPT_A]

    devices = jax.devices()[:NCORES]
    mesh = Mesh(np.asarray(devices), ("core",))
    nargs = 5 + len(WNAMES)
    in_specs = (PartitionSpec("core"),) * nargs
    out_specs = PartitionSpec("core")
    return jax.jit(shard_map(body, mesh=mesh, in_specs=in_specs,
                             out_specs=out_specs, check_rep=False))


def _pad_slice(arr, lo, hi):
    n = arr.shape[0]
    lo_pad = max(0, -lo)
    hi_pad = max(0, hi - n)
    core = arr[max(lo, 0):min(hi, n)]
    if lo_pad or hi_pad:
        pad = [(lo_pad, hi_pad)] + [(0, 0)] * (arr.ndim - 1)
        core = np.pad(core, pad)
    return core


def _concat_inputs(inputs):
    """Build the axis-0-concatenated global arrays for shard_map."""
    a = np.asarray(inputs['a'], np.float32)
    ef = np.asarray(inputs['extra_feats'], np.float32)
    plm = np.asarray(inputs['p_lm'], np.float32)
    am = np.asarray(inputs['atom_mask'], np.float32)
    idx = np.asarray(inputs['atom_to_token_idx'], np.int32)

    a_l, ef_l, plm_l, am_l, idx_l = [], [], [], [], []
    for c in range(NCORES):
        b, ws = divmod(c, WSLICES)
        a0 = ws * KEPT_A - HALO_A
        a1 = ws * KEPT_A + KEPT_A + HALO_A
        w0 = ws * KEPT_W - HALO_W
        w1 = ws * KEPT_W + KEPT_W + HALO_W
        a_l.append(a[b])
        ef_l.append(_pad_slice(ef[b], a0, a1))
        plm_l.append(_pad_slice(plm[b], w0, w1))
        am_l.append(_pad_slice(am[b], a0, a1))
        idx_l.append(np.clip(_pad_slice(idx[b], a0, a1), 0, NTOK - 1))
    cat = [np.concatenate(a_l, 0), np.concatenate(ef_l, 0),
           np.concatenate(plm_l, 0), np.concatenate(am_l, 0),
           np.concatenate(idx_l, 0)]
    for k in WNAMES:
        w = np.asarray(inputs[k], np.float32)
        cat.append(np.broadcast_to(w[None], (NCORES,) + w.shape).reshape(
            (NCORES,) + w.shape))
    return cat


def kernel(**inputs) -> np.ndarray:
    global _jitted
    if _jitted is None:
        _jitted = _build_spmd_fn()
    out = np.asarray(_jitted(*_concat_inputs(inputs)))   # [8*KEPT_A, 3]
    out = out.reshape(NCORES, KEPT_A, 3)
    full = np.empty((B, NATOM, 3), np.float32)
    for c in range(NCORES):
        b, ws = divmod(c, WSLICES)
        full[b, ws * KEPT_A:(ws + 1) * KEPT_A] = out[c]
    return full

